# revision 48
# baseline (speedup 1.0000x reference)
"""Trainium2 Bass kernel for ExhaustiveBiaffineNERDecoder.

reference semantics:
  masked BatchNorm(features) -> FFN(768->4096) + ReLU
  -> reshape [B,T,16,128,2] -> start/end features
  -> scores[b,l,s,e] = sum_d start[b,s,l,d]*end[b,e,l,d] + label_bias[l]
  -> spans_mask = triu & mask & mask  (pure boolean, computed on host)

Sharding: 2-D grid over (sample-groups x label-groups), default 4x2: each core
handles 2 samples x 8 labels. BN stats are global over the batch: each core
computes (sum, sum-of-squares) over its local samples and the per-feature
pairs are AllReduced across the 8 cores (6 KB collective).

Numerics: everything stream-level is bf16 (inputs cast on host, activations
and the scores output quantized on device); accumulation stays fp32 in PSUM
and the BN-stats reduction accumulates fp32. Measured end-to-end error vs the
fp32 reference is ~4e-3 scale-relative (gate is 2e-2). bf16 halves every DMA
transfer (in: x 1.6 MB + W 3.1 MB, out: scores 8.4 MB per core), taking the
DMA engines well below the PE roofline that previously paced the kernel.

Layout trick: ff_w rows are permuted on the host to [label, start/end, d_out]
order and the whole weight is transposed to [768, 4096]. The FFN then directly
produces h^T tiles [128 d_out x T tokens] per (label, start/end) -- exactly
the stationary/moving operands the biaffine matmul needs, so there are no
on-device transposes at all.

Schedule: software-pipelined one label deep -- FFN matmuls for label l are
emitted before the biaffine matmuls of label l-1, so the ReLU drain of h(l)
overlaps the next label's FFN and the PE never waits. Score tiles drain
PSUM->SBUF (with +label_bias, fp32->bf16) alternating scalar/vector, and fly
out as one 256 KB HWDGE DMA per (label, sample) alternating the sync and
scalar queues. Weights are loaded once and stay resident in SBUF.

Timing methodology (test.py): the whole pipeline is replayed U=16 times per
For_i iteration; bodies inside an iteration software-pipeline via tile-pool
rotation (the per-iteration all-engine barrier amortizes /U), and the
wall-clock delta between two iteration counts divided by the body delta
cancels dispatch overhead and first-iteration cold effects. PE busy/body is
~54 us (256 matmuls at the measured 206 ns sustained rate), which is the
bf16 compute roofline for this decomposition.
"""

import os

import ml_dtypes
import numpy as np

import concourse.bacc as bacc
import concourse.mybir as mybir
import concourse.tile as tile
from concourse import bass_utils

F32 = mybir.dt.float32
BF16 = mybir.dt.bfloat16
NPBF = ml_dtypes.bfloat16
AF = mybir.ActivationFunctionType
ALU = mybir.AluOpType

B, T, D = 8, 512, 768
NL, LD = 16, 128
O = NL * LD * 2  # 4096
DC = D // 128  # 6 contraction chunks
BN_EPS = 1e-5
N_CORES = 8

_CACHE = {}
last_run_info = None  # BassKernelResults of the most recent run (for profiling)


def _shard():
    s = os.environ.get("BIAFFINE_SHARD", "4x2")
    sg, lg = (int(v) for v in s.split("x"))
    assert sg * lg == N_CORES
    return sg, lg


def _stats_mode():
    return os.environ.get("BIAFFINE_STATS", "ar")  # "ar" (AllReduce) or "local"


def _build_nc(stats_mode="ar", bench_loop=1, loop_scope="body", sg=4, lg=2, unroll=1):
    spc = B // sg  # samples per core
    lpc = NL // lg  # labels per core
    TL = spc * T  # local tokens
    OL = lpc * LD * 2  # local FFN output cols
    NH = TL // 512  # moving-dim halves (PSUM bank holds 512 fp32)
    QW = 512
    NQ = OL // QW

    nc = bacc.Bacc("TRN2", target_bir_lowering=False, debug=False, num_devices=N_CORES)

    wT = nc.dram_tensor("wT", [D, OL], BF16, kind="ExternalInput")
    xto = nc.dram_tensor("xto", [D, TL], BF16, kind="ExternalInput")
    maskf = nc.dram_tensor("maskf", [1, TL], BF16, kind="ExternalInput")
    gamma = nc.dram_tensor("gamma", [D], F32, kind="ExternalInput")
    beta = nc.dram_tensor("beta", [D], F32, kind="ExternalInput")
    ffb = nc.dram_tensor("ffb", [OL], F32, kind="ExternalInput")
    lbias = nc.dram_tensor("lbias", [1, lpc], F32, kind="ExternalInput")
    # tile-native layout: every output DMA writes one fully contiguous
    # 256 KB block (the [spc,lpc,T,T] layout scattered 1 KB segments at
    # 128 KB strides -- HBM page thrashing, amplified across 8 cores);
    # the host un-permutes during the fp32 upconvert
    scores = nc.dram_tensor(
        "scores", [lpc, spc, 128, 4, T], BF16, kind="ExternalOutput"
    )
    if stats_mode == "ar":
        cc_in = nc.dram_tensor("cc_in", [128, DC, 2], F32, kind="Internal")
        cc_out = nc.dram_tensor(
            "cc_out", [128, DC, 2], F32, kind="Internal", addr_space="Shared"
        )

    with tile.TileContext(nc) as tc:
        with (
            tc.tile_pool(name="const", bufs=1) as const,
            tc.tile_pool(name="wp", bufs=2) as wp,
            tc.tile_pool(name="xstat", bufs=2) as xstat,
            tc.tile_pool(name="stats", bufs=2) as stats,
            tc.tile_pool(name="xn", bufs=2) as xnp,
            tc.tile_pool(name="tmp", bufs=2) as tmpp,
            tc.tile_pool(name="h", bufs=4) as hp,
            tc.tile_pool(name="sc", bufs=4) as scp,
            tc.tile_pool(name="ph", bufs=2, space="PSUM") as psum_h,
            tc.tile_pool(name="psc", bufs=3, space="PSUM") as psum_s,
        ):
            # ---- constants ----
            g_t = const.tile([128, DC], F32, tag="g")
            nc.scalar.dma_start(out=g_t[:], in_=gamma[:].rearrange("(c p) -> p c", p=128))
            bt_t = const.tile([128, DC], F32, tag="bt")
            nc.scalar.dma_start(out=bt_t[:], in_=beta[:].rearrange("(c p) -> p c", p=128))
            # local ff_b in [d_out, label, se] order (matches W row permutation)
            ffb_t = const.tile([128, lpc, 2], F32, tag="ffb")
            nc.scalar.dma_start(
                out=ffb_t[:],
                in_=ffb[:].rearrange("(l d s) -> d l s", l=lpc, d=128, s=2),
            )
            lb_t = const.tile([128, lpc], F32, tag="lb")
            nc.scalar.dma_start(out=lb_t[:], in_=lbias[:].partition_broadcast(128))
            mask_t = const.tile([128, TL], BF16, tag="mask")
            nc.scalar.dma_start(out=mask_t[:], in_=maskf[:].partition_broadcast(128))
            eps_t = const.tile([128, 1], F32, tag="eps")
            nc.vector.memset(eps_t[:], BN_EPS)
            # warm the activation table set before the fold chain needs
            # Sqrt (a cold set-load costs ~2.7us in the critical path)
            warm_t = const.tile([128, 1], F32, tag="warm")
            nc.scalar.activation(out=warm_t[:], in_=eps_t[:], func=AF.Sqrt)

            # weight blocks are loop-invariant parameters: load once, keep
            # resident in SBUF (j-interleaved column order so block q covers
            # labels 2q..2q+1)
            wT_p = wT[:].rearrange("(c p) o -> p c o", p=128)
            w_blocks = []
            for q in range(NQ):
                w_b = wp.tile([128, DC, QW], BF16, tag=f"wq{q}")
                nc.sync.dma_start(out=w_b[:], in_=wT_p[:, :, q * QW : (q + 1) * QW])
                w_blocks.append(w_b)

            # everything per-iteration lives in load_strips() + prefix() +
            # _emit_main() so the bench modes can wrap either just the main
            # compute ("body") or the whole pipeline ("full") in an on-device
            # repeat loop. load_strips is emitted one body EARLY in the
            # unrolled loop so the issues aren't queued behind the previous
            # body's output DMAs on the sync queue.
            def load_strips():
                xto_c = xto[:].rearrange("(c p) t -> c p t", p=128)
                xo_tiles = []
                for c in range(DC):
                    xo_t = xstat.tile([128, TL], BF16, tag=f"xo{c}")
                    nc.sync.dma_start(out=xo_t[:], in_=xto_c[c])
                    xo_tiles.append(xo_t)
                return xo_tiles

            def prefix(xo_tiles, collective_ok=True):
                # ---- BN statistics: linear partials (sum, sum of squares)
                # in one AllReduce payload; sums over cores combine exactly.
                # Sum-reduces split DVE/gpsimd, squares fused on scalar, so
                # the three engines chew strips in parallel as they land.
                # per-engine accumulator tiles: a single shared tile would
                # false-serialize the three stat lanes across engines
                send_sum = stats.tile([128, DC], F32, tag="send_sum")
                send_sq = stats.tile([128, DC], F32, tag="send_sq")
                for c in range(DC):
                    jk = tmpp.tile([128, TL], BF16, tag="jkv")
                    # tensor_scalar w/ fused accumulator: 4x DVE mode (vs the
                    # 1x-only TensorReduce) for the same sum
                    nc.vector.tensor_scalar(
                        jk[:],
                        xo_tiles[c][:],
                        0.0,
                        None,
                        ALU.add,
                        ALU.add,
                        accum_out=send_sum[:, c : c + 1],
                    )
                    # Square is a 1-ULP filler in the sqrt set, so these
                    # never force an activation-table switch
                    sq = tmpp.tile([128, TL], BF16, tag="jks")
                    nc.scalar.activation(
                        out=sq[:],
                        in_=xo_tiles[c][:],
                        func=AF.Square,
                        accum_out=send_sq[:, c : c + 1],
                    )
                g_sum = stats.tile([128, DC, 2], F32, tag="gsum")
                if collective_ok:
                    nc.scalar.dma_start(out=cc_in[:, :, 0], in_=send_sum[:])
                    nc.scalar.dma_start(out=cc_in[:, :, 1], in_=send_sq[:])
                    nc.gpsimd.collective_compute(
                        "AllReduce",
                        ALU.add,
                        replica_groups=[list(range(N_CORES))],
                        ins=[cc_in[:]],
                        outs=[cc_out[:]],
                    )
                    nc.scalar.dma_start(out=g_sum[:], in_=cc_out[:])
                else:
                    # timing-only stand-in (collectives can't sit in a loop)
                    nc.scalar.mul(g_sum[:, :, 0], send_sum[:], float(N_CORES))
                    nc.scalar.mul(g_sum[:, :, 1], send_sq[:], float(N_CORES))
                # fold to per-partition scale a / bias b
                inv = 1.0 / (lg * B * T)
                mean6 = tmpp.tile([128, DC], F32, tag="mean")
                nc.vector.tensor_scalar_mul(mean6[:], g_sum[:, :, 0], inv)
                msq6 = tmpp.tile([128, DC], F32, tag="msq")
                nc.vector.tensor_mul(msq6[:], mean6[:], mean6[:])
                var6 = tmpp.tile([128, DC], F32, tag="var")
                nc.vector.scalar_tensor_tensor(
                    var6[:], g_sum[:, :, 1], inv, msq6[:], ALU.mult, ALU.subtract
                )
                sd6 = tmpp.tile([128, DC], F32, tag="sd")
                nc.scalar.activation(
                    out=sd6[:], in_=var6[:], func=AF.Sqrt, bias=eps_t[:], scale=1.0
                )
                rq6 = tmpp.tile([128, DC], F32, tag="rq")
                nc.vector.reciprocal(out=rq6[:], in_=sd6[:])
                a6 = stats.tile([128, DC], F32, tag="a6")
                nc.vector.tensor_mul(a6[:], rq6[:], g_t[:])
                t6m = tmpp.tile([128, DC], F32, tag="t6m")
                nc.vector.tensor_mul(t6m[:], mean6[:], a6[:])
                b6 = stats.tile([128, DC], F32, tag="b6")
                nc.vector.tensor_sub(b6[:], bt_t[:], t6m[:])

                # ---- normalized+masked activations, bf16, full-width ops
                # spread over scalar/DVE/gpsimd ----
                # norms split scalar/DVE, masks trail on DVE (gpsimd element-
                # wise is 2-3x slower than DVE -- keep it off this path)
                xn_tiles = []
                for c in range(DC):
                    t3 = tmpp.tile([128, TL], BF16, tag=f"t3{c % 2}")
                    if c % 2 == 0:
                        nc.scalar.activation(
                            out=t3[:],
                            in_=xo_tiles[c][:],
                            func=AF.Identity,
                            bias=b6[:, c : c + 1],
                            scale=a6[:, c : c + 1],
                        )
                    else:
                        nc.vector.tensor_scalar(
                            t3[:],
                            xo_tiles[c][:],
                            a6[:, c : c + 1],
                            b6[:, c : c + 1],
                            ALU.mult,
                            ALU.add,
                        )
                    xn_c = xnp.tile([128, TL], BF16, tag=f"xn{c}")
                    nc.vector.tensor_tensor(xn_c[:], t3[:], mask_t[:], ALU.mult)
                    xn_tiles.append(xn_c)
                return xn_tiles

            def main_body(w_blocks, xn_tiles, tail_hook=None):
                _emit_main(
                    nc, w_blocks, xn_tiles, ffb_t, lb_t, hp, scp, psum_h, psum_s,
                    scores, spc, lpc, TL, NH, QW, tail_hook=tail_hook,
                )

            cok = stats_mode == "ar"
            if bench_loop > 1 and loop_scope == "full":
                # For_i puts an all-engine barrier at each iteration; U
                # unrolled bodies inside one iteration pipeline freely via
                # pool rotation, so the barrier+head resync amortizes /U.
                # Strips for body u+1 are issued at the top of body u.
                # the NEXT body's strips+stats chain is emitted just before
                # the last label's biaffine drains, so on the in-order scalar
                # and DVE queues it executes ahead of those final drains and
                # the next body's first matmul fires with no normalize stall
                with tc.For_i(0, bench_loop, 1) as _i:
                    xo_next = load_strips()
                    for _u in range(unroll):
                        xo_cur = xo_next
                        if _u + 1 < unroll:
                            xo_next = load_strips()
                        xn = prefix(xo_cur, collective_ok=False)
                        main_body(w_blocks, xn)
            elif bench_loop > 1:
                xn = prefix(load_strips(), collective_ok=cok)
                with tc.For_i(0, bench_loop, 1) as _i:
                    main_body(w_blocks, xn)
            else:
                xn = prefix(load_strips(), collective_ok=cok)
                main_body(w_blocks, xn)

    nc.compile()
    return nc


def _emit_main(
    nc, w_blocks, xn_tiles, ffb_t, lb_t, hp, scp, psum_h, psum_s, scores,
    spc, lpc, TL, NH, QW, tail_hook=None,
):
    h_of = {}  # label -> (h_start, h_end)
    drain_idx = 0
    dma_idx = 0

    def emit_ffn_se(l, se):
        j = l * 2 + se
        q, jj = divmod(j * 128, QW)
        h_t = hp.tile([128, TL], BF16, tag="h")
        for half in range(NH):
            ph = psum_h.tile([128, 512], mybir.dt.float32, tag="ph")
            for c in range(len(xn_tiles)):
                nc.tensor.matmul(
                    ph[:],
                    w_blocks[q][:, c, jj : jj + 128],
                    xn_tiles[c][:, half * 512 : (half + 1) * 512],
                    start=(c == 0),
                    stop=(c == len(xn_tiles) - 1),
                )
            nc.scalar.activation(
                out=h_t[:, half * 512 : (half + 1) * 512],
                in_=ph[:],
                func=AF.Relu,
                bias=ffb_t[:, l, se : se + 1],
                scale=1.0,
            )
        h_of.setdefault(l, []).append(h_t)

    out_mode = os.environ.get("BIAFFINE_OUT", "split")
    assert out_mode in ("split", "noout"), "coal needs the old scores layout"
    # coal mode: scores[b] flattens to [(l i p), e] with l-stride == 4*i-stride,
    # so a label PAIR for one sample is a single contiguous-ish 3-D AP -- one
    # 512 KB DMA instead of two 256 KB ones.
    pair_tiles = {}

    def emit_biaffine_b(l, b):
        nonlocal drain_idx, dma_idx
        h_s, h_e = h_of[l]
        last_l = l == lpc - 1
        lhalf = l % 2
        if True:
            last_b = last_l and b == spc - 1
            if out_mode == "coal":
                if lhalf == 0:
                    pair_tiles[b] = scp.tile(
                        [128, 8, T], BF16, tag="sc", name=f"scpair{b}"
                    )
                sc_t = pair_tiles[b]
                so = lhalf * 4
            else:
                sc_t = scp.tile([128, 4, T], BF16, tag="sc")
                so = 0
            out_ap = scores[l, b]
            for half in range(2):
                psc = psum_s.tile([128, 2, T], mybir.dt.float32, tag="psc")
                for i2 in range(2):
                    i = half * 2 + i2
                    nc.tensor.matmul(
                        psc[:, i2, :],
                        h_s[:, b * T + i * 128 : b * T + (i + 1) * 128],
                        h_e[:, b * T : (b + 1) * T],
                        start=True,
                        stop=True,
                    )
                if last_b:
                    # last tile: quarter drains in parallel on scalar+DVE and
                    # quarter DMAs so the final transfer is tiny
                    for i2 in range(2):
                        i = half * 2 + i2
                        eng_add = nc.scalar.add if i2 == 0 else (
                            nc.vector.tensor_scalar_add
                        )
                        eng_add(
                            sc_t[:, so + i : so + i + 1, :],
                            psc[:, i2 : i2 + 1, :],
                            lb_t[:, l : l + 1],
                        )
                        if out_mode != "noout":
                            nc.sync.dma_start(
                                out=out_ap[:, i : i + 1, :],
                                in_=sc_t[:, so + i : so + i + 1, :],
                            )
                    continue
                # drain PSUM -> bf16 SBUF with +label_bias; ~3/8 on scalar
                lo = so + half * 2
                if drain_idx % 8 in (0, 3, 6):
                    nc.scalar.add(
                        sc_t[:, lo : lo + 2, :], psc[:], lb_t[:, l : l + 1]
                    )
                else:
                    nc.vector.tensor_scalar_add(
                        sc_t[:, lo : lo + 2, :], psc[:], lb_t[:, l : l + 1]
                    )
                drain_idx += 1
            if out_mode == "split" and not last_b:
                # alternate the two HWDGE rings (sync/scalar queues); the
                # gpsimd SWDGE path costs ~1.1us of Pool sequencer per issue
                # and measured slower
                eng = nc.sync if dma_idx % 2 == 0 else nc.scalar
                eng.dma_start(out=out_ap[:], in_=sc_t[:])
                dma_idx += 1
            elif out_mode == "coal" and lhalf == 1:
                pair_ap = scores[b].rearrange("l (i p) e -> p (l i) e", p=128)
                li0 = (l - 1) * 4
                eng = nc.sync if dma_idx % 2 == 0 else nc.scalar
                if last_b:
                    # final sample's odd-label quarters already flew out above
                    eng.dma_start(
                        out=pair_ap[:, li0 : li0 + 4, :], in_=sc_t[:, 0:4, :]
                    )
                else:
                    eng.dma_start(
                        out=pair_ap[:, li0 : li0 + 8, :], in_=sc_t[:]
                    )
                dma_idx += 1

    # biaffine bursts for label l-1 are interleaved between the two FFN
    # groups of label l: each pair of psc drains gets a ~2.5us FFN window to
    # complete, removing the psc-WAR stalls that paced back-to-back bursts
    for l in range(lpc):
        emit_ffn_se(l, 0)
        if l > 0:
            emit_biaffine_b(l - 1, 0)
        emit_ffn_se(l, 1)
        if l > 0:
            for b in range(1, spc):
                emit_biaffine_b(l - 1, b)
    if tail_hook is not None:
        tail_hook()
    for b in range(spc):
        emit_biaffine_b(lpc - 1, b)


def _get_nc(
    stats_mode=None, bench_loop=1, loop_scope="body", sg=None, lg=None, unroll=1
):
    if stats_mode is None:
        stats_mode = _stats_mode()
    if sg is None:
        sg, lg = _shard()
    key = ("nc", stats_mode, bench_loop, loop_scope, sg, lg, unroll)
    if key not in _CACHE:
        _CACHE[key] = _build_nc(stats_mode, bench_loop, loop_scope, sg, lg, unroll)
    return _CACHE[key]


def make_in_maps(features, mask_b, bn_gamma, bn_beta, ff_w, ff_b, label_bias, sg, lg):
    spc = B // sg
    lpc = NL // lg
    TL = spc * T
    OL = lpc * LD * 2

    xtf = np.ascontiguousarray(features.reshape(B * T, D).T).astype(NPBF)  # [768, B*T]
    wT = np.ascontiguousarray(
        ff_w.reshape(NL, LD, 2, D).transpose(3, 0, 2, 1).reshape(D, O)
    ).astype(NPBF)  # [768, (l,se,d_out)]
    maskf = mask_b.astype(NPBF).reshape(B * T)

    in_maps = []
    for i in range(sg):
        for k in range(lg):
            in_maps.append(
                {
                    "wT": np.ascontiguousarray(wT[:, k * OL : (k + 1) * OL]),
                    "xto": np.ascontiguousarray(xtf[:, i * TL : (i + 1) * TL]),
                    "maskf": np.ascontiguousarray(
                        maskf[i * TL : (i + 1) * TL].reshape(1, TL)
                    ),
                    "gamma": bn_gamma,
                    "beta": bn_beta,
                    "ffb": np.ascontiguousarray(ff_b[k * OL : (k + 1) * OL]),
                    "lbias": np.ascontiguousarray(
                        label_bias[k * lpc : (k + 1) * lpc].reshape(1, lpc)
                    ),
                }
            )
    return in_maps


def kernel(features, mask, bn_gamma, bn_beta, ff_w, ff_b, label_bias):
    global last_run_info
    features = np.asarray(features, dtype=np.float32)
    mask_b = np.asarray(mask).astype(bool)
    bn_gamma = np.asarray(bn_gamma, dtype=np.float32)
    bn_beta = np.asarray(bn_beta, dtype=np.float32)
    ff_w = np.asarray(ff_w, dtype=np.float32)
    ff_b = np.asarray(ff_b, dtype=np.float32)
    label_bias = np.asarray(label_bias, dtype=np.float32)

    sg, lg = _shard()
    spc = B // sg
    lpc = NL // lg
    nc = _get_nc(_stats_mode(), sg=sg, lg=lg)
    in_maps = make_in_maps(
        features, mask_b, bn_gamma, bn_beta, ff_w, ff_b, label_bias, sg, lg
    )

    res = bass_utils.run_bass_kernel_spmd(
        nc,
        in_maps,
        core_ids=list(range(N_CORES)),
        trace=bool(os.environ.get("BIAFFINE_TRACE")),
    )
    last_run_info = res
    scores = np.empty((B, NL, T, T), dtype=np.float32)
    for i in range(sg):
        for k in range(lg):
            core = i * lg + k
            blk = res.results[core]["scores"]  # [lpc, spc, 128, 4, T] bf16
            # device tile layout -> [spc, lpc, s=(i*128+p), e]
            blk = np.transpose(blk, (1, 0, 3, 2, 4)).reshape(spc, lpc, T, T)
            scores[i * spc : (i + 1) * spc, k * lpc : (k + 1) * lpc] = blk.astype(
                np.float32
            )

    # span mask: pure boolean broadcast, no FLOPs
    triu = np.triu(np.ones((T, T), dtype=bool))
    spans = triu[None, None] & mask_b[:, None, :, None] & mask_b[:, None, None, :]
    spans = np.broadcast_to(spans, scores.shape)
    return scores, spans
